# revision 37
# baseline (speedup 1.0000x reference)
"""Trainium2 Bass/Tile kernel for nn_Attention_3418793967804.

8-way data parallel over batch (1 batch per NeuronCore). Per core:
qkv 1x1 conv (+folded BN), 4-head attention over 2304 positions,
depthwise 3x3 conv on v, residual add, final 1x1 conv (+folded BN).

Layout: S^T score tiles (keys on partitions) via row-packed K=32 bf16
matmuls; softmax exp is split between the scalar engine (exact spline)
and the vector engine (Schraudolph int-bitcast approximation writing
int32 whose bits form the f32 exp) so the two engines drain score
tiles in parallel; attention-value matmuls use a [V^T | ones-col]
65-wide stationary per head so one matmul yields the numerator (rows
0-63) and the softmax denominator (row 64); V^T blocks are computed
directly on the PE as x^T @ Wv matmuls (no transposes, no bias -- the
V bias passes linearly through softmax and the final conv, so it is
folded into the c2 bias); normalization stages the denominator to one
sbuf partition, reciprocals it, and gpsimd-broadcasts it across 64
partitions; the depthwise 3x3 conv stays on the PE as 9 diagonal
matmuls; dw/c2 transition chunks are emitted inside the NEXT i-chunk's
loop (after its first two score tiles) so the scalar engine restarts
immediately at i-chunk boundaries while the PE interleaves the
transition work; a zero-stationary warmup burst at kernel start keeps
the PE HAM clock-gate warm through the initial DMA wait.
"""
import numpy as np

import concourse.bass as bass
import concourse.mybir as mybir
import concourse.tile as tile
from concourse import bacc

F32 = mybir.dt.float32
F32R = mybir.dt.float32r
I16 = mybir.dt.int16
BF16 = mybir.dt.bfloat16
EXP = mybir.ActivationFunctionType.Exp
MULT = mybir.AluOpType.mult
ADD = mybir.AluOpType.add

CH = 256
HW = 2304
H = W = 48
NH = 4
DK = 32
DH = 64
SCALE = float(DK) ** -0.5
EPS = 1e-3

IC_SIZES = [512, 512, 512, 512, 256]
IC_STARTS = [0, 512, 1024, 1536, 2048]
JB = 18          # 2304 / 128 j-blocks
QN = 384         # qkv/dwconv spatial chunk = 8 rows of 48
NQ = HW // QN    # 6
PW = 50          # padded width/height

# Schraudolph exp in bf16: bits(int16(s*EA + EB)) ~= exp(SCALE*s) as bf16,
# max rel err ~3.3% (B chosen robust to the f32->int16 rounding mode).
# Output stays bf16 so AV matmuls keep uniform bf16 operands.
EA = float(SCALE * np.log2(np.e) * 2.0 ** 7)
EB = 16250.75
# score tiles with (jb*2+hp) % 7 in {0,3} drain on the vector engine (2/7)


def build_consts(qkv_w, qkv_g, qkv_b, qkv_m, qkv_v, c1_w, c1_g, c1_b, c1_m, c1_v,
                 c2_w, c2_g, c2_b, c2_m, c2_v):
    """Fold BN into conv weights and pack into device-layout numpy arrays."""
    f = np.float32
    sq = qkv_g / np.sqrt(qkv_v + EPS)
    Wq = (qkv_w[:, :, 0, 0] * sq[:, None]).astype(f)       # (512, 256)
    bq = (qkv_b - qkv_m * sq).astype(f)                    # (512,)
    s1 = c1_g / np.sqrt(c1_v + EPS)
    W1 = (c1_w[:, 0, :, :] * s1[:, None, None]).astype(f)  # (256, 3, 3)
    b1 = (c1_b - c1_m * s1).astype(f)
    s2 = c2_g / np.sqrt(c2_v + EPS)
    W2 = (c2_w[:, :, 0, 0] * s2[:, None]).astype(f)        # (256, 256)
    b2 = (c2_b - c2_m * s2).astype(f)

    # qkv output channel permutation: cols 0-127 Q_all (h*32+dk), 128-255 K_all,
    # 256-511 v in natural c = h*64+d order
    perm = np.zeros(512, dtype=np.int64)
    for col in range(128):
        h, dk = col // 32, col % 32
        perm[col] = 128 * h + dk
        perm[128 + col] = 128 * h + 32 + dk
    for col in range(256):
        h, d = col // 64, col % 64
        perm[256 + col] = 128 * h + 64 + d
    wt = np.ascontiguousarray(Wq[perm].T)                  # (256 ic, 512 col)
    bqkv = np.zeros((128, 4), f)
    for occ in range(4):
        bqkv[:, occ] = bq[perm[occ * 128:(occ + 1) * 128]]

    # depthwise conv diagonal stationaries: block (chunk, tap) at cols
    # (chunk*9+tap)*128, diag entries W1[chunk*128 + c, tap]
    diag = np.zeros((128, 18 * 128), f)
    for chunk in range(2):
        for tap in range(9):
            di, dj = tap // 3, tap % 3
            base = (chunk * 9 + tap) * 128
            idx = np.arange(128)
            diag[idx, base + idx] = W1[chunk * 128 + idx, di, dj]

    w2t = np.ascontiguousarray(W2.T)                        # (256 c, 256 oc)
    # the attention numerator uses unbiased V (softmax weights sum to 1, so
    # the V bias adds bv to every attention output); both that bv and the dw
    # bias b1 pass linearly through the final conv: fold them there
    bv = bq[perm[256:512]]                                  # (256,) c order
    b2e = (b2 + W2 @ (b1 + bv)).astype(f)
    b2p = np.stack([b2e[0:128], b2e[128:256]], axis=1).astype(f)  # (128, 2)
    return dict(wt=wt, bqkv=bqkv, diag=diag, w2t=w2t, b2p=b2p)


def build_nc(debug=False):
    nc = bacc.Bacc("TRN2", target_bir_lowering=False, debug=False,
                   enable_asserts=True, num_devices=8)
    dp = {}
    def din(name, shape, dt=F32):
        dp[name] = nc.dram_tensor(name, list(shape), dt, kind="ExternalInput").ap()
    din("x", (256, HW), F32R)
    din("wt", (256, 512), F32R)
    din("bqkv", (128, 4))
    din("diag", (128, 18 * 128), F32R)
    din("w2t", (256, 256), F32R)
    din("b2p", (128, 2))
    out_d = nc.dram_tensor("out", [256, HW], F32, kind="ExternalOutput").ap()
    dbg = {}
    if debug:
        for name, shape in [("dq", (128, HW)), ("dk", (128, HW)),
                            ("dvt", (128, JB * 512)),
                            ("dy0", (128, HW)), ("dy1", (128, HW)),
                            ("dot0", (128, HW)), ("dot1", (128, HW)),
                            ("dvp0", (128, PW * PW)), ("dvp1", (128, PW * PW))]:
            dbg[name] = nc.dram_tensor(name, list(shape), F32, kind="ExternalOutput").ap()

    with tile.TileContext(nc) as tc:
        build_body(nc, tc, dp, out_d, dbg)
    nc.compile()
    return nc


def build_body(nc, tc, dp, out_d, dbg):
    from contextlib import ExitStack
    with ExitStack() as ctx:
        ep = ctx.enter_context
        wpool = ep(tc.tile_pool(name="w", bufs=1))
        xpool = ep(tc.tile_pool(name="x", bufs=1))
        qkpool = ep(tc.tile_pool(name="qk", bufs=1))
        vtpool = ep(tc.tile_pool(name="vt", bufs=1))
        vppool = ep(tc.tile_pool(name="vp", bufs=1))
        ypool = ep(tc.tile_pool(name="y", bufs=1))
        ppool = ep(tc.tile_pool(name="pp", bufs=10))
        npool = ep(tc.tile_pool(name="np", bufs=2))
        otpool = ep(tc.tile_pool(name="ot", bufs=1))
        obpool = ep(tc.tile_pool(name="ob", bufs=3))

        # --- weights & inputs ---
        wt_r = [wpool.tile([128, 512], F32R, tag=f"wt{c}", name=f"wt{c}") for c in range(2)]
        diag_r = wpool.tile([128, 18 * 128], F32R, tag="diag", name="diag")
        w2t_r = [wpool.tile([128, 256], F32R, tag=f"w2t{c}", name=f"w2t{c}") for c in range(2)]
        bq_f = wpool.tile([128, 4], F32, tag="bqf", name="bqf")
        b2_f = wpool.tile([128, 2], F32, tag="b2f", name="b2f")
        wu = wpool.tile([128, 128], BF16, tag="wu", name="wu")
        x_r = [xpool.tile([128, HW], F32R, tag=f"x{c}", name=f"x{c}") for c in range(2)]

        # critical-path loads first: bias + first x quarter of both halves +
        # the Q/K half of wt (everything early qkv matmuls need), then the
        # rest; bulky late-use weights (diag/w2t) after the attention starts
        nc.sync.dma_start(bq_f[:], dp["bqkv"][:])
        qr_sl = lambda qr: slice(qr * (HW // 4), (qr + 1) * (HW // 4))
        for c in range(2):
            nc.sync.dma_start(x_r[c][:, qr_sl(0)], dp["x"][128 * c:128 * (c + 1), qr_sl(0)])
        for c in range(2):
            nc.sync.dma_start(wt_r[c][:, 0:256], dp["wt"][128 * c:128 * (c + 1), 0:256])
        for c in range(2):
            nc.sync.dma_start(wt_r[c][:, 256:512], dp["wt"][128 * c:128 * (c + 1), 256:512])
        for qr in range(1, 4):
            for c in range(2):
                nc.sync.dma_start(x_r[c][:, qr_sl(qr)], dp["x"][128 * c:128 * (c + 1), qr_sl(qr)])
        nc.sync.dma_start(b2_f[:], dp["b2p"][:])

        Q = qkpool.tile([128, HW], BF16, tag="Q", name="Q")
        K = qkpool.tile([128, HW], BF16, tag="K", name="K")
        # per (jb, head) stationary block: [V^T_h (64 cols) | ones (1 col)];
        # output row 64 collects the softmax denominator at 1/128th of the
        # array power a 64-wide ones block would burn
        VTO = vtpool.tile([128, JB * 512], BF16, tag="VTO", name="VTO")
        vto4 = VTO[:].rearrange("p (b k) -> p b k", k=128)
        nc.gpsimd.memset(vto4[:, :, 64:65], 1.0)
        if dbg:
            nc.gpsimd.memset(vto4[:, :, 65:128], 0.0)
        vp = [vppool.tile([128, PW * PW], F32R, tag=f"vp{c}", name=f"vp{c}") for c in range(2)]
        for c in range(2):
            # zero only the 1-wide pad border; the interior is fully
            # overwritten by the qkv V evacuation
            vp3i = vp[c][:].bitcast(F32).rearrange("p (r w) -> p r w", w=PW)
            nc.gpsimd.memset(vp3i[:, 0:1, :], 0.0)
            nc.gpsimd.memset(vp3i[:, 49:50, :], 0.0)
            nc.gpsimd.memset(vp3i[:, 1:49, 0:1], 0.0)
            nc.gpsimd.memset(vp3i[:, 1:49, 49:50], 0.0)
        y_all = [ypool.tile([128, HW], F32, tag=f"y{c}", name=f"y{c}") for c in range(2)]
        ot = [otpool.tile([128, HW], F32R, tag=f"ot{c}", name=f"ot{c}") for c in range(2)]
        # raw depthwise-conv output staging (SBUF): decouples the dw psum
        # slot handoff from the y-chain so AV matmuls restart early
        dwb = [otpool.tile([128, HW], F32, tag=f"dwb{c}", name=f"dwb{c}") for c in range(2)]

        with tc.tile_pool(name="psS", bufs=2, space="PSUM") as psS, \
             tc.tile_pool(name="psU", bufs=1, space="PSUM") as psU:

            # HAM warmup: ~4.5us of zero-stationary matmuls at kernel start
            # (pure DMA-wait time otherwise) so the PE clock-gate is already
            # at K=8/8 when the first real matmul issues
            nc.gpsimd.memset(wu[:], 0.0)
            psW = psS.tile([128, 1024], F32, tag="s2", name="warm")
            for _ in range(44):
                nc.tensor.matmul(psW[:, 0:128], wu[:], wu[:], start=True, stop=True)

            def emit_qkv(occ, g):
                # one 384-wide chunk of the qkv projection for output group occ
                ps = psS.tile([128, 1024], F32, tag="s2", name="s2")
                sl = slice(g * QN, (g + 1) * QN)
                for c in range(2):
                    nc.tensor.matmul(
                        ps[:, 0:QN], wt_r[c][:, occ * 128:(occ + 1) * 128],
                        x_r[c][:, sl], start=(c == 0), stop=(c == 1))
                bias_ap = bq_f[:, occ:occ + 1]
                if occ == 0:
                    nc.vector.tensor_scalar_add(Q[:, sl], ps[:, 0:QN], bias_ap)
                elif occ == 1:
                    nc.vector.tensor_scalar_add(K[:, sl], ps[:, 0:QN], bias_ap)
                else:
                    c = occ - 2
                    vp3 = vp[c][:].rearrange("p (r w) -> p r w", w=PW)
                    dst = vp3[:, 1 + 8 * g:1 + 8 * g + 8, 1:49]
                    srcp = ps[:, 0:QN].rearrange("p (r w) -> p r w", w=48)
                    nc.vector.tensor_scalar_add(dst, srcp, bias_ap)

            def emit_dw(c, g, slot=None):
                # depthwise 3x3 conv chunk via 9 diagonal matmuls; the raw
                # conv is copied straight out to SBUF (no y dependency, so
                # the psum slot frees immediately) and the attention-output
                # add happens on the otherwise-idle gpsimd engine
                ps = slot() if slot else psS.tile([128, 1024], F32, tag="s2", name="s2")
                vp3 = vp[c][:].rearrange("p (r w) -> p r w", w=PW)
                for tap in range(9):
                    di, dj = tap // 3, tap % 3
                    mov = vp3[:, 8 * g + di:8 * g + di + 8, dj:dj + 48]
                    nc.tensor.matmul(
                        ps[:, 0:QN], diag_r[:, (c * 9 + tap) * 128:(c * 9 + tap + 1) * 128],
                        mov, start=(tap == 0), stop=(tap == 8))
                sl = slice(g * QN, (g + 1) * QN)
                nc.vector.tensor_copy(dwb[c][:, sl], ps[:, 0:QN])
                nc.gpsimd.tensor_add(ot[c][:, sl], dwb[c][:, sl], y_all[c][:, sl])

            # minimal qkv pre-work: just what the first attention iterations
            # need; the rest interleaves into ic 0 (Q/K) and ic 1 (V, only
            # needed by the dw conv image) via qkv_sched
            for occ, g in [(0, 0), (0, 1), (1, 0), (2, 0), (3, 0)]:
                emit_qkv(occ, g)
            qkv_sched = {
                0: {0: [(1, 1)], 1: [(0, 2)], 2: [(1, 2)], 4: [(0, 3)],
                    5: [(1, 3)], 7: [(0, 4)], 8: [(1, 4)], 10: [(0, 5)],
                    11: [(1, 5)]},
                1: {0: [(2, 1)], 1: [(3, 1)], 4: [(2, 2)], 5: [(3, 2)],
                    7: [(2, 3)], 8: [(3, 3)], 10: [(2, 4)], 11: [(3, 4)],
                    13: [(2, 5)], 14: [(3, 5)]},
            }

            def emit_vt_pair(jb):
                # V^T blocks for jb, jb+1 directly on the PE: x^T @ Wv (2
                # accum matmuls per block over the input-channel halves into
                # the two banks of one S tile), then one strided DVE copy per
                # block splits the 4 heads into their [V^T | ones] stationary
                # blocks. No bias (folded into b2e).
                ps = psS.tile([128, 1024], F32, tag="s2", name="s2")
                for k2 in range(2):
                    jj = jb + k2
                    jsl = slice(jj * 128, (jj + 1) * 128)
                    for c in range(2):
                        nc.tensor.matmul(ps[:, k2 * 512:k2 * 512 + 256],
                                         x_r[c][:, jsl], wt_r[c][:, 256:512],
                                         start=(c == 0), stop=(c == 1))
                for k2 in range(2):
                    jj = jb + k2
                    src = ps[:, k2 * 512:k2 * 512 + 256].rearrange(
                        "p (h d) -> p h d", d=64)
                    nc.vector.tensor_copy(vto4[:, jj * 4:jj * 4 + 4, 0:64], src)

            def emit_c2(occ, k, slot=None):
                n2 = IC_SIZES[k]
                isl2 = slice(IC_STARTS[k], IC_STARTS[k] + n2)
                ps = slot() if slot else psS.tile([128, 1024], F32, tag="s2", name="s2")
                for c in range(2):
                    nc.tensor.matmul(ps[:, 0:n2],
                                     w2t_r[c][:, occ * 128:(occ + 1) * 128],
                                     ot[c][:, isl2], start=(c == 0), stop=(c == 1))
                ob = obpool.tile([128, 512], F32, tag="ob", name="ob")
                nc.vector.tensor_scalar_add(ob[:, 0:n2], ps[:, 0:n2],
                                            b2_f[:, occ:occ + 1])
                nc.sync.dma_start(out_d[occ * 128:(occ + 1) * 128, isl2], ob[:, 0:n2])

            # ic -> dw/c2 chunks scheduled after that ic's normalization
            # (which frees the U psum slots); they are EMITTED inside the
            # next ic's jb loop (after its first two score tiles) so the
            # scalar engine restarts right away while the PE interleaves
            # the transition work. deps: dw(c,g) needs y cols <=
            # IC_STARTS[ic+1], c2(occ,k) needs its dw coverage.
            trans_sched = {
                0: [('dw', 0, 0), ('dw', 1, 0)],
                1: [('dw', 0, 1), ('dw', 1, 1), ('c2', 0, 0), ('c2', 1, 0)],
                2: [('dw', 0, 2), ('dw', 1, 2), ('dw', 0, 3), ('dw', 1, 3)],
                3: [('dw', 0, 4), ('dw', 1, 4), ('c2', 0, 1), ('c2', 1, 1),
                    ('c2', 0, 2), ('c2', 1, 2)],
            }
            pending_dw = []
            pending_c2 = []

            def emit_pending_dw():
                # boundary dw chunks ride the U psum slots (freed by the
                # previous ic's U-copies); emitted before this ic's first AV
                slots = [lambda h=h: psU.tile([128, 512], F32, tag=f"Uh{h}",
                                              name=f"aux{h}") for h in range(4)]
                for i, (c, g) in enumerate(pending_dw):
                    emit_dw(c, g, slot=slots[i % 4])
                pending_dw.clear()

            for ic in range(5):
                n = IC_SIZES[ic]
                i0 = IC_STARTS[ic]
                isl = slice(i0, i0 + n)
                # created lazily at first AV use so the slot rotation orders
                # them AFTER the previous ic's pending dw/c2 aux tiles
                Uh = []

                def get_Uh():
                    if not Uh:
                        Uh.extend(psU.tile([128, 512], F32, tag=f"Uh{h}",
                                           name=f"Uh{h}") for h in range(4))
                    return Uh

                def emit_qk_exp(jb):
                    jsl = slice(jb * 128, (jb + 1) * 128)
                    p2s = []
                    for hp in range(2):
                        s2 = psS.tile([128, 1024], F32, tag="s2", name="s2")
                        for hh in range(2):
                            h = 2 * hp + hh
                            nc.tensor.matmul(
                                s2[:, hh * 512:hh * 512 + n],
                                K[32 * h:32 * (h + 1), jsl],
                                Q[32 * h:32 * (h + 1), isl],
                                start=True, stop=True, tile_position=(32 * h, 0))
                        p2 = ppool.tile([128, 1024], BF16, tag="p2", name="p2")
                        # jb < 2 stays on the scalar engine so the vector
                        # engine is free for the boundary U-copies and never
                        # gates the S-slot rotation there
                        if jb >= 2 and (jb * 2 + hp) % 7 in (0, 2, 4):
                            # vector-engine drain: Schraudolph exp via int16
                            # convert; the int bits ARE the bf16 approximation
                            d3 = p2[:].bitcast(I16).rearrange("p (a b) -> p a b", b=512)[:, :, 0:n]
                            s3 = s2[:].rearrange("p (a b) -> p a b", b=512)[:, :, 0:n]
                            nc.vector.tensor_scalar(
                                out=d3, in0=s3, scalar1=EA, scalar2=EB,
                                op0=MULT, op1=ADD)
                        else:
                            if n == 512:
                                nc.scalar.activation(p2[:], s2[:], EXP, scale=SCALE)
                            else:
                                s3 = s2[:].rearrange("p (a b) -> p a b", b=512)[:, :, 0:n]
                                p3 = p2[:].rearrange("p (a b) -> p a b", b=512)[:, :, 0:n]
                                nc.scalar.activation(p3, s3, EXP, scale=SCALE)
                        p2s.append(p2)
                    return p2s

                def emit_av(jb, p2s):
                    # one matmul per head: [V^T | ones-col] stationary gives
                    # numerator rows 0-63 and the denominator on row 64
                    U = get_Uh()
                    for h in range(4):
                        hp, hh = h // 2, h % 2
                        mov = p2s[hp][:, hh * 512:hh * 512 + n]
                        nc.tensor.matmul(
                            U[h][0:65, 0:n],
                            VTO[:, jb * 512 + 128 * h:jb * 512 + 128 * h + 65],
                            mov, start=(jb == 0), stop=(jb == JB - 1))

                # software pipeline: boundary dw chunks are emitted at jb 2
                # (their U slots free after the previous ic's fast U-copies)
                # and AV starts at jb 3; boundary c2 chunks ride S slots late
                # in the ic (jb 6, 8, ...) once ot is ready
                av_start = 3 if pending_dw else 1
                avq = []
                for jb in range(JB):
                    p2s = emit_qk_exp(jb)
                    avq.append((jb, p2s))
                    if ic <= 1:
                        # qkv chunks BEFORE the pending dw block: dw(c,g)
                        # reads vp rows from the g+1 chunk, so the V writes
                        # must be emitted first
                        if ic == 0 and jb % 2 == 0 and jb < 18:
                            emit_vt_pair(jb)
                        for occ, g in qkv_sched.get(ic, {}).get(jb, ()):
                            emit_qkv(occ, g)
                    if jb == 2 and pending_dw:
                        emit_pending_dw()
                    if jb >= av_start:
                        while avq and avq[0][0] <= jb - 1:
                            emit_av(*avq.pop(0))
                    if pending_c2 and jb >= 6 and jb % 2 == 0:
                        emit_c2(*pending_c2.pop(0))
                    if (ic, jb) == (0, 5):
                        # late-use weight loads, clear of the early burst
                        for c in range(2):
                            nc.sync.dma_start(w2t_r[c][:], dp["w2t"][128 * c:128 * (c + 1), :])
                        nc.sync.dma_start(diag_r[:], dp["diag"][:])
                while avq:
                    emit_av(*avq.pop(0))

                # normalization: y_h = U[0:64] / U[64]. Denominator rows and
                # numerator blocks are copied out to SBUF first (numerators
                # split across the scalar and vector engines) so the U psum
                # slots free in ~1.5us and the next ic's boundary dw + AV
                # matmuls restart immediately; the reciprocal runs on the
                # vector engine, and the broadcast + multiply run on the
                # otherwise-idle gpsimd engine
                Ubs, rbs = [], []
                for h in range(4):
                    # per-head buffers: all four copies are emitted before
                    # the first recip, so a shared 2-deep rotation would be
                    # overwritten in FIFO order before it is read
                    rb = npool.tile([128, 512], F32, tag=f"rb{h}", name=f"rb{h}", bufs=1)
                    nc.vector.tensor_copy(rb[0:1, 0:n], get_Uh()[h][64:65, 0:n])
                    rbs.append(rb)
                for h in range(4):
                    ub = npool.tile([128, 512], F32, tag=f"ub{h}", name=f"ub{h}", bufs=1)
                    if h % 2 == 0:
                        nc.scalar.copy(ub[0:64, 0:n], get_Uh()[h][0:64, 0:n])
                    else:
                        nc.vector.tensor_copy(ub[0:64, 0:n], get_Uh()[h][0:64, 0:n])
                    Ubs.append(ub)
                for h in range(4):
                    hp, hh = h // 2, h % 2
                    rr = npool.tile([128, 512], F32, tag=f"rr{h}", name=f"rr{h}", bufs=1)
                    bc = npool.tile([128, 512], F32, tag=f"bc{h}", name=f"bc{h}", bufs=1)
                    nc.vector.reciprocal_approx_fast(rr[0:1, 0:n], rbs[h][0:1, 0:n])
                    nc.gpsimd.partition_broadcast(bc[0:64, 0:n], rr[0:1, 0:n])
                    nc.gpsimd.tensor_mul(y_all[hp][64 * hh:64 * hh + 64, isl],
                                         Ubs[h][0:64, 0:n], bc[0:64, 0:n])

                for kind, a, b_ in trans_sched.get(ic, []):
                    (pending_dw if kind == 'dw' else pending_c2).append((a, b_))

            # --- tail: remaining dw + c2 chunks ---
            pending_dw.extend([(0, 5), (1, 5)])
            emit_pending_dw()
            while pending_c2:
                emit_c2(*pending_c2.pop(0))
            for occ in range(2):
                for k in (3, 4):
                    emit_c2(occ, k)

        if dbg:
            nc.gpsimd.dma_start(dbg["dq"][:], Q[:])
            nc.gpsimd.dma_start(dbg["dk"][:], K[:])
            nc.gpsimd.dma_start(dbg["dvt"][:], VTO[:])
            nc.sync.dma_start(dbg["dy0"][:], y_all[0][:])
            nc.sync.dma_start(dbg["dy1"][:], y_all[1][:])
            nc.sync.dma_start(dbg["dot0"][:], ot[0][:].bitcast(F32))
            nc.sync.dma_start(dbg["dot1"][:], ot[1][:].bitcast(F32))
            nc.sync.dma_start(dbg["dvp0"][:], vp[0][:].bitcast(F32))
            nc.sync.dma_start(dbg["dvp1"][:], vp[1][:].bitcast(F32))


def make_in_maps(x_full, consts):
    maps = []
    for b in range(8):
        m = dict(consts)
        m["x"] = np.ascontiguousarray(x_full[b].reshape(256, HW), dtype=np.float32)
        maps.append(m)
    return maps

_CACHED = {}


def _get_nc():
    if 'nc' not in _CACHED:
        _CACHED['nc'] = build_nc(debug=False)
    return _CACHED['nc']


def kernel(**inputs):
    """Full (unsharded) inputs -> full output (8, 256, 48, 48) float32."""
    from concourse.bass_utils import run_bass_kernel_spmd

    x = np.asarray(inputs['x'], dtype=np.float32)
    consts = build_consts(**{k: np.asarray(v) for k, v in inputs.items()
                             if k != 'x'})
    in_maps = make_in_maps(x, consts)
    nc = _get_nc()
    try:
        res = run_bass_kernel_spmd(nc, in_maps, list(range(8)))
    except Exception:
        # first execution after a fresh compile occasionally hits a
        # transient device error; one retry clears it
        res = run_bass_kernel_spmd(nc, in_maps, list(range(8)))
    out = np.stack([res.results[b]['out'].reshape(256, 48, 48)
                    for b in range(8)])
    return out.astype(np.float32)


# revision 41
# speedup vs baseline: 1.3782x; 1.3782x over previous
"""Trainium2 Bass/Tile kernel for nn_Attention_3418793967804.

8-way data parallel over batch (1 batch per NeuronCore). Per core:
qkv 1x1 conv (+folded BN), 4-head attention over 2304 positions,
depthwise 3x3 conv on v, residual add, final 1x1 conv (+folded BN).

Layout: S^T score tiles (keys on partitions) via row-packed K=32 bf16
matmuls; softmax exp is split between the scalar engine (exact spline)
and the vector engine (Schraudolph int-bitcast approximation writing
int32 whose bits form the f32 exp) so the two engines drain score
tiles in parallel; attention-value matmuls use a [V^T | ones-col]
65-wide stationary per head so one matmul yields the numerator (rows
0-63) and the softmax denominator (row 64); V^T blocks are computed
directly on the PE as x^T @ Wv matmuls (no transposes, no bias -- the
V bias passes linearly through softmax and the final conv, so it is
folded into the c2 bias); normalization stages the denominator to one
sbuf partition, reciprocals it, and gpsimd-broadcasts it across 64
partitions; the depthwise 3x3 conv stays on the PE as 9 diagonal
matmuls; dw/c2 transition chunks are emitted inside the NEXT i-chunk's
loop (after its first two score tiles) so the scalar engine restarts
immediately at i-chunk boundaries while the PE interleaves the
transition work; a zero-stationary warmup burst at kernel start keeps
the PE HAM clock-gate warm through the initial DMA wait.
"""
import numpy as np

import concourse.bass as bass
import concourse.mybir as mybir
import concourse.tile as tile
from concourse import bacc

F32 = mybir.dt.float32
F32R = mybir.dt.float32r
I16 = mybir.dt.int16
BF16 = mybir.dt.bfloat16
EXP = mybir.ActivationFunctionType.Exp
MULT = mybir.AluOpType.mult
ADD = mybir.AluOpType.add

CH = 256
HW = 2304
H = W = 48
NH = 4
DK = 32
DH = 64
SCALE = float(DK) ** -0.5
EPS = 1e-3

IC_SIZES = [512, 512, 512, 512, 256]
IC_STARTS = [0, 512, 1024, 1536, 2048]
JB = 18          # 2304 / 128 j-blocks
QN = 384         # qkv/dwconv spatial chunk = 8 rows of 48
NQ = HW // QN    # 6
PW = 50          # padded width/height

# Schraudolph exp in bf16: bits(int16(s*EA + EB)) ~= exp(SCALE*s) as bf16,
# max rel err ~3.3% (B chosen robust to the f32->int16 rounding mode).
# Output stays bf16 so AV matmuls keep uniform bf16 operands.
EA = float(SCALE * np.log2(np.e) * 2.0 ** 7)
EB = 16250.75
# score tiles with (jb*2+hp) % 7 in {0,3} drain on the vector engine (2/7)


def build_consts(qkv_w, qkv_g, qkv_b, qkv_m, qkv_v, c1_w, c1_g, c1_b, c1_m, c1_v,
                 c2_w, c2_g, c2_b, c2_m, c2_v):
    """Fold BN into conv weights and pack into device-layout numpy arrays."""
    f = np.float32
    sq = qkv_g / np.sqrt(qkv_v + EPS)
    Wq = (qkv_w[:, :, 0, 0] * sq[:, None]).astype(f)       # (512, 256)
    bq = (qkv_b - qkv_m * sq).astype(f)                    # (512,)
    s1 = c1_g / np.sqrt(c1_v + EPS)
    W1 = (c1_w[:, 0, :, :] * s1[:, None, None]).astype(f)  # (256, 3, 3)
    b1 = (c1_b - c1_m * s1).astype(f)
    s2 = c2_g / np.sqrt(c2_v + EPS)
    W2 = (c2_w[:, :, 0, 0] * s2[:, None]).astype(f)        # (256, 256)
    b2 = (c2_b - c2_m * s2).astype(f)

    # qkv output channel permutation: cols 0-127 Q_all (h*32+dk), 128-255 K_all,
    # 256-511 v in natural c = h*64+d order
    perm = np.zeros(512, dtype=np.int64)
    for col in range(128):
        h, dk = col // 32, col % 32
        perm[col] = 128 * h + dk
        perm[128 + col] = 128 * h + 32 + dk
    for col in range(256):
        h, d = col // 64, col % 64
        perm[256 + col] = 128 * h + 64 + d
    wt = np.ascontiguousarray(Wq[perm].T)                  # (256 ic, 512 col)
    bqkv = np.zeros((128, 4), f)
    for occ in range(4):
        bqkv[:, occ] = bq[perm[occ * 128:(occ + 1) * 128]]

    # depthwise conv diagonal stationaries: block (chunk, tap) at cols
    # (chunk*9+tap)*128, diag entries W1[chunk*128 + c, tap]
    diag = np.zeros((128, 18 * 128), f)
    for chunk in range(2):
        for tap in range(9):
            di, dj = tap // 3, tap % 3
            base = (chunk * 9 + tap) * 128
            idx = np.arange(128)
            diag[idx, base + idx] = W1[chunk * 128 + idx, di, dj]

    w2t = np.ascontiguousarray(W2.T)                        # (256 c, 256 oc)
    # the attention numerator uses unbiased V (softmax weights sum to 1, so
    # the V bias adds bv to every attention output); both that bv and the dw
    # bias b1 pass linearly through the final conv: fold them there
    bv = bq[perm[256:512]]                                  # (256,) c order
    b2e = (b2 + W2 @ (b1 + bv)).astype(f)
    b2p = np.stack([b2e[0:128], b2e[128:256]], axis=1).astype(f)  # (128, 2)
    return dict(wt=wt, bqkv=bqkv, diag=diag, w2t=w2t, b2p=b2p)


def build_nc(debug=False):
    nc = bacc.Bacc("TRN2", target_bir_lowering=False, debug=False,
                   enable_asserts=True, num_devices=8)
    dp = {}
    def din(name, shape, dt=F32):
        dp[name] = nc.dram_tensor(name, list(shape), dt, kind="ExternalInput").ap()
    din("x", (256, HW), F32R)
    din("wt", (256, 512), F32R)
    din("bqkv", (128, 4))
    din("diag", (128, 18 * 128), F32R)
    din("w2t", (256, 256), F32R)
    din("b2p", (128, 2))
    out_d = nc.dram_tensor("out", [256, HW], F32, kind="ExternalOutput").ap()
    dbg = {}
    if debug:
        for name, shape in [("dq", (128, HW)), ("dk", (128, HW)),
                            ("dvt", (128, JB * 512)),
                            ("dy0", (128, HW)), ("dy1", (128, HW)),
                            ("dot0", (128, HW)), ("dot1", (128, HW)),
                            ("dvp0", (128, PW * PW)), ("dvp1", (128, PW * PW))]:
            dbg[name] = nc.dram_tensor(name, list(shape), F32, kind="ExternalOutput").ap()

    with tile.TileContext(nc) as tc:
        build_body(nc, tc, dp, out_d, dbg)
    nc.compile()
    return nc


def build_body(nc, tc, dp, out_d, dbg):
    from contextlib import ExitStack
    with ExitStack() as ctx:
        ep = ctx.enter_context
        wpool = ep(tc.tile_pool(name="w", bufs=1))
        xpool = ep(tc.tile_pool(name="x", bufs=1))
        qkpool = ep(tc.tile_pool(name="qk", bufs=1))
        vtpool = ep(tc.tile_pool(name="vt", bufs=1))
        vppool = ep(tc.tile_pool(name="vp", bufs=1))
        ypool = ep(tc.tile_pool(name="y", bufs=1))
        ppool = ep(tc.tile_pool(name="pp", bufs=10))
        npool = ep(tc.tile_pool(name="np", bufs=2))
        otpool = ep(tc.tile_pool(name="ot", bufs=1))
        obpool = ep(tc.tile_pool(name="ob", bufs=3))

        # --- weights & inputs ---
        wt_r = [wpool.tile([128, 512], F32R, tag=f"wt{c}", name=f"wt{c}") for c in range(2)]
        diag_r = wpool.tile([128, 18 * 128], F32R, tag="diag", name="diag")
        w2t_r = [wpool.tile([128, 256], F32R, tag=f"w2t{c}", name=f"w2t{c}") for c in range(2)]
        bq_f = wpool.tile([128, 4], F32, tag="bqf", name="bqf")
        b2_f = wpool.tile([128, 2], F32, tag="b2f", name="b2f")
        wu = wpool.tile([128, 128], BF16, tag="wu", name="wu")
        x_r = [xpool.tile([128, HW], F32R, tag=f"x{c}", name=f"x{c}") for c in range(2)]

        # critical-path loads first: bias + first x quarter of both halves +
        # the Q/K half of wt (everything early qkv matmuls need), then the
        # rest; bulky late-use weights (diag/w2t) after the attention starts
        nc.sync.dma_start(bq_f[:], dp["bqkv"][:])
        qr_sl = lambda qr: slice(qr * (HW // 4), (qr + 1) * (HW // 4))
        for c in range(2):
            nc.sync.dma_start(x_r[c][:, qr_sl(0)], dp["x"][128 * c:128 * (c + 1), qr_sl(0)])
        for c in range(2):
            nc.sync.dma_start(wt_r[c][:, 0:256], dp["wt"][128 * c:128 * (c + 1), 0:256])
        for c in range(2):
            nc.sync.dma_start(wt_r[c][:, 256:512], dp["wt"][128 * c:128 * (c + 1), 256:512])
        for qr in range(1, 4):
            for c in range(2):
                nc.sync.dma_start(x_r[c][:, qr_sl(qr)], dp["x"][128 * c:128 * (c + 1), qr_sl(qr)])
        nc.sync.dma_start(b2_f[:], dp["b2p"][:])

        Q = qkpool.tile([128, HW], BF16, tag="Q", name="Q")
        K = qkpool.tile([128, HW], BF16, tag="K", name="K")
        # per (jb, head) stationary block: [V^T_h (64 cols) | ones (1 col)];
        # output row 64 collects the softmax denominator at 1/128th of the
        # array power a 64-wide ones block would burn
        VTO = vtpool.tile([128, JB * 512], BF16, tag="VTO", name="VTO")
        vto4 = VTO[:].rearrange("p (b k) -> p b k", k=128)
        nc.gpsimd.memset(vto4[:, :, 64:65], 1.0)
        if dbg:
            nc.gpsimd.memset(vto4[:, :, 65:128], 0.0)
        vp = [vppool.tile([128, PW * PW], F32R, tag=f"vp{c}", name=f"vp{c}") for c in range(2)]
        for c in range(2):
            # zero only the 1-wide pad border; the interior is fully
            # overwritten by the qkv V evacuation
            vp3i = vp[c][:].bitcast(F32).rearrange("p (r w) -> p r w", w=PW)
            nc.gpsimd.memset(vp3i[:, 0:1, :], 0.0)
            nc.gpsimd.memset(vp3i[:, 49:50, :], 0.0)
            nc.gpsimd.memset(vp3i[:, 1:49, 0:1], 0.0)
            nc.gpsimd.memset(vp3i[:, 1:49, 49:50], 0.0)
        y_all = [ypool.tile([128, HW], F32, tag=f"y{c}", name=f"y{c}") for c in range(2)]
        ot = [otpool.tile([128, HW], F32R, tag=f"ot{c}", name=f"ot{c}") for c in range(2)]
        # raw depthwise-conv output staging (SBUF): decouples the dw psum
        # slot handoff from the y-chain so AV matmuls restart early
        dwb = [otpool.tile([128, HW], F32, tag=f"dwb{c}", name=f"dwb{c}") for c in range(2)]

        with tc.tile_pool(name="psS", bufs=2, space="PSUM") as psS, \
             tc.tile_pool(name="psU", bufs=1, space="PSUM") as psU:

            # HAM warmup: ~4.5us of zero-stationary matmuls at kernel start
            # (pure DMA-wait time otherwise) so the PE clock-gate is already
            # at K=8/8 when the first real matmul issues
            nc.gpsimd.memset(wu[:], 0.0)
            psW = psS.tile([128, 1024], F32, tag="s2", name="warm")
            for _ in range(44):
                nc.tensor.matmul(psW[:, 0:128], wu[:], wu[:], start=True, stop=True)

            def emit_qkv(occ, g):
                # one 384-wide chunk of the qkv projection for output group occ
                ps = psS.tile([128, 1024], F32, tag="s2", name="s2")
                sl = slice(g * QN, (g + 1) * QN)
                for c in range(2):
                    nc.tensor.matmul(
                        ps[:, 0:QN], wt_r[c][:, occ * 128:(occ + 1) * 128],
                        x_r[c][:, sl], start=(c == 0), stop=(c == 1))
                bias_ap = bq_f[:, occ:occ + 1]
                if occ == 0:
                    nc.vector.tensor_scalar_add(Q[:, sl], ps[:, 0:QN], bias_ap)
                elif occ == 1:
                    nc.vector.tensor_scalar_add(K[:, sl], ps[:, 0:QN], bias_ap)
                else:
                    c = occ - 2
                    vp3 = vp[c][:].rearrange("p (r w) -> p r w", w=PW)
                    dst = vp3[:, 1 + 8 * g:1 + 8 * g + 8, 1:49]
                    srcp = ps[:, 0:QN].rearrange("p (r w) -> p r w", w=48)
                    nc.vector.tensor_scalar_add(dst, srcp, bias_ap)

            def emit_dw(c, g, slot=None):
                # depthwise 3x3 conv chunk via 9 diagonal matmuls; the raw
                # conv is copied straight out to SBUF (no y dependency, so
                # the psum slot frees immediately) and the attention-output
                # add happens on the otherwise-idle gpsimd engine
                ps = slot() if slot else psS.tile([128, 1024], F32, tag="s2", name="s2")
                vp3 = vp[c][:].rearrange("p (r w) -> p r w", w=PW)
                for tap in range(9):
                    di, dj = tap // 3, tap % 3
                    mov = vp3[:, 8 * g + di:8 * g + di + 8, dj:dj + 48]
                    nc.tensor.matmul(
                        ps[:, 0:QN], diag_r[:, (c * 9 + tap) * 128:(c * 9 + tap + 1) * 128],
                        mov, start=(tap == 0), stop=(tap == 8))
                sl = slice(g * QN, (g + 1) * QN)
                nc.vector.tensor_copy(dwb[c][:, sl], ps[:, 0:QN])
                nc.vector.tensor_add(ot[c][:, sl], dwb[c][:, sl], y_all[c][:, sl])

            # minimal qkv pre-work: just what the first attention iterations
            # need; the rest interleaves into ic 0 (Q/K) and ic 1 (V, only
            # needed by the dw conv image) via qkv_sched
            for occ, g in [(0, 0), (0, 1), (1, 0), (2, 0), (3, 0)]:
                emit_qkv(occ, g)
            qkv_sched = {
                0: {0: [(1, 1)], 1: [(0, 2)], 2: [(1, 2)], 4: [(0, 3)],
                    5: [(1, 3)], 7: [(0, 4)], 8: [(1, 4)], 10: [(0, 5)],
                    11: [(1, 5)], 14: [(2, 1)], 16: [(3, 1)]},
                1: {4: [(2, 2)], 5: [(3, 2)],
                    7: [(2, 3)], 8: [(3, 3)], 10: [(2, 4)], 11: [(3, 4)],
                    13: [(2, 5)], 14: [(3, 5)]},
            }

            def emit_vt_pair(jb):
                # V^T blocks for jb, jb+1 directly on the PE: x^T @ Wv (2
                # accum matmuls per block over the input-channel halves into
                # the two banks of one S tile), then one strided DVE copy per
                # block splits the 4 heads into their [V^T | ones] stationary
                # blocks. No bias (folded into b2e).
                ps = psS.tile([128, 1024], F32, tag="s2", name="s2")
                for k2 in range(2):
                    jj = jb + k2
                    jsl = slice(jj * 128, (jj + 1) * 128)
                    for c in range(2):
                        nc.tensor.matmul(ps[:, k2 * 512:k2 * 512 + 256],
                                         x_r[c][:, jsl], wt_r[c][:, 256:512],
                                         start=(c == 0), stop=(c == 1))
                for k2 in range(2):
                    jj = jb + k2
                    src = ps[:, k2 * 512:k2 * 512 + 256].rearrange(
                        "p (h d) -> p h d", d=64)
                    nc.vector.tensor_copy(vto4[:, jj * 4:jj * 4 + 4, 0:64], src)

            def emit_c2(occ, k, slot=None):
                n2 = IC_SIZES[k]
                isl2 = slice(IC_STARTS[k], IC_STARTS[k] + n2)
                ps = slot() if slot else psS.tile([128, 1024], F32, tag="s2", name="s2")
                for c in range(2):
                    nc.tensor.matmul(ps[:, 0:n2],
                                     w2t_r[c][:, occ * 128:(occ + 1) * 128],
                                     ot[c][:, isl2], start=(c == 0), stop=(c == 1))
                ob = obpool.tile([128, 512], F32, tag="ob", name="ob")
                nc.vector.tensor_scalar_add(ob[:, 0:n2], ps[:, 0:n2],
                                            b2_f[:, occ:occ + 1])
                nc.sync.dma_start(out_d[occ * 128:(occ + 1) * 128, isl2], ob[:, 0:n2])

            # ic -> dw/c2 chunks scheduled after that ic's normalization
            # (which frees the U psum slots); they are EMITTED inside the
            # next ic's jb loop (after its first two score tiles) so the
            # scalar engine restarts right away while the PE interleaves
            # the transition work. deps: dw(c,g) needs y cols <=
            # IC_STARTS[ic+1], c2(occ,k) needs its dw coverage.
            trans_sched = {
                0: [('dw', 0, 0), ('dw', 1, 0)],
                1: [('dw', 0, 1), ('dw', 1, 1), ('c2', 0, 0), ('c2', 1, 0)],
                2: [('dw', 0, 2), ('dw', 1, 2), ('dw', 0, 3), ('dw', 1, 3)],
                3: [('dw', 0, 4), ('dw', 1, 4), ('c2', 0, 1), ('c2', 1, 1),
                    ('c2', 0, 2), ('c2', 1, 2)],
            }
            pending_dw = []
            pending_c2 = []

            def emit_pending_dw():
                # boundary dw chunks ride the U psum slots (freed by the
                # previous ic's U-copies); emitted before this ic's first AV
                slots = [lambda h=h: psU.tile([128, 512], F32, tag=f"Uh{h}",
                                              name=f"aux{h}") for h in range(4)]
                for i, (c, g) in enumerate(pending_dw):
                    emit_dw(c, g, slot=slots[i % 4])
                pending_dw.clear()

            for ic in range(5):
                n = IC_SIZES[ic]
                i0 = IC_STARTS[ic]
                isl = slice(i0, i0 + n)
                # created lazily at first AV use so the slot rotation orders
                # them AFTER the previous ic's pending dw/c2 aux tiles
                Uh = []

                def get_Uh():
                    if not Uh:
                        Uh.extend(psU.tile([128, 512], F32, tag=f"Uh{h}",
                                           name=f"Uh{h}") for h in range(4))
                    return Uh

                def emit_qk_exp(jb):
                    jsl = slice(jb * 128, (jb + 1) * 128)
                    p2s = []
                    for hp in range(2):
                        s2 = psS.tile([128, 1024], F32, tag="s2", name="s2")
                        for hh in range(2):
                            h = 2 * hp + hh
                            nc.tensor.matmul(
                                s2[:, hh * 512:hh * 512 + n],
                                K[32 * h:32 * (h + 1), jsl],
                                Q[32 * h:32 * (h + 1), isl],
                                start=True, stop=True, tile_position=(32 * h, 0))
                        p2 = ppool.tile([128, 1024], BF16, tag="p2", name="p2")
                        # jb < 2 stays on the scalar engine so the vector
                        # engine is free for the boundary U-copies and never
                        # gates the S-slot rotation there
                        if jb >= 2 and (jb * 2 + hp) % 7 in (0, 4):
                            # vector-engine drain: Schraudolph exp via int16
                            # convert; the int bits ARE the bf16 approximation
                            d3 = p2[:].bitcast(I16).rearrange("p (a b) -> p a b", b=512)[:, :, 0:n]
                            s3 = s2[:].rearrange("p (a b) -> p a b", b=512)[:, :, 0:n]
                            nc.vector.tensor_scalar(
                                out=d3, in0=s3, scalar1=EA, scalar2=EB,
                                op0=MULT, op1=ADD)
                        else:
                            if n == 512:
                                nc.scalar.activation(p2[:], s2[:], EXP, scale=SCALE)
                            else:
                                s3 = s2[:].rearrange("p (a b) -> p a b", b=512)[:, :, 0:n]
                                p3 = p2[:].rearrange("p (a b) -> p a b", b=512)[:, :, 0:n]
                                nc.scalar.activation(p3, s3, EXP, scale=SCALE)
                        p2s.append(p2)
                    return p2s

                def emit_av(jb, p2s):
                    # one matmul per head: [V^T | ones-col] stationary gives
                    # numerator rows 0-63 and the denominator on row 64
                    U = get_Uh()
                    for h in range(4):
                        hp, hh = h // 2, h % 2
                        mov = p2s[hp][:, hh * 512:hh * 512 + n]
                        nc.tensor.matmul(
                            U[h][0:65, 0:n],
                            VTO[:, jb * 512 + 128 * h:jb * 512 + 128 * h + 65],
                            mov, start=(jb == 0), stop=(jb == JB - 1))

                # software pipeline: boundary dw chunks are emitted at jb 2
                # (their U slots free after the previous ic's fast U-copies)
                # and AV starts at jb 3; boundary c2 chunks ride S slots late
                # in the ic (jb 6, 8, ...) once ot is ready
                av_start = 3 if pending_dw else 1
                avq = []
                for jb in range(JB):
                    p2s = emit_qk_exp(jb)
                    avq.append((jb, p2s))
                    if ic <= 1:
                        # qkv chunks BEFORE the pending dw block: dw(c,g)
                        # reads vp rows from the g+1 chunk, so the V writes
                        # must be emitted first
                        if ic == 0 and jb % 2 == 0 and jb < 18:
                            emit_vt_pair(jb)
                        for occ, g in qkv_sched.get(ic, {}).get(jb, ()):
                            emit_qkv(occ, g)
                    if jb == 2 and pending_dw:
                        emit_pending_dw()
                    if jb >= av_start:
                        while avq and avq[0][0] <= jb - 1:
                            emit_av(*avq.pop(0))
                    if pending_c2 and jb >= 6 and jb % 2 == 0:
                        emit_c2(*pending_c2.pop(0))
                    if (ic, jb) == (0, 5):
                        # late-use weight loads, clear of the early burst
                        for c in range(2):
                            nc.sync.dma_start(w2t_r[c][:], dp["w2t"][128 * c:128 * (c + 1), :])
                        nc.sync.dma_start(diag_r[:], dp["diag"][:])
                while avq:
                    emit_av(*avq.pop(0))

                # normalization: y_h = U[0:64] / U[64]. Denominator rows and
                # numerator blocks are copied out to SBUF first (numerators
                # split across the scalar and vector engines) so the U psum
                # slots free in ~1.5us and the next ic's boundary dw + AV
                # matmuls restart immediately; the reciprocal runs on the
                # vector engine, and the broadcast + multiply run on the
                # otherwise-idle gpsimd engine
                Ubs, rbs = [], []
                for h in range(4):
                    # per-head buffers: all four copies are emitted before
                    # the first recip, so a shared 2-deep rotation would be
                    # overwritten in FIFO order before it is read
                    rb = npool.tile([128, 512], F32, tag=f"rb{h}", name=f"rb{h}", bufs=1)
                    nc.vector.tensor_copy(rb[0:1, 0:n], get_Uh()[h][64:65, 0:n])
                    rbs.append(rb)
                for h in range(4):
                    ub = npool.tile([128, 512], F32, tag=f"ub{h}", name=f"ub{h}", bufs=1)
                    nc.scalar.copy(ub[0:64, 0:n], get_Uh()[h][0:64, 0:n])
                    Ubs.append(ub)
                for h in range(4):
                    hp, hh = h // 2, h % 2
                    rr = npool.tile([128, 512], F32, tag=f"rr{h}", name=f"rr{h}", bufs=1)
                    bc = npool.tile([128, 512], F32, tag=f"bc{h}", name=f"bc{h}", bufs=1)
                    nc.vector.reciprocal_approx_fast(rr[0:1, 0:n], rbs[h][0:1, 0:n])
                    nc.gpsimd.partition_broadcast(bc[0:64, 0:n], rr[0:1, 0:n])
                    nc.vector.tensor_mul(y_all[hp][64 * hh:64 * hh + 64, isl],
                                         Ubs[h][0:64, 0:n], bc[0:64, 0:n])

                for kind, a, b_ in trans_sched.get(ic, []):
                    (pending_dw if kind == 'dw' else pending_c2).append((a, b_))

            # --- tail: remaining dw + c2 chunks ---
            pending_dw.extend([(0, 5), (1, 5)])
            emit_pending_dw()
            while pending_c2:
                emit_c2(*pending_c2.pop(0))
            for occ in range(2):
                for k in (3, 4):
                    emit_c2(occ, k)

        if dbg:
            nc.gpsimd.dma_start(dbg["dq"][:], Q[:])
            nc.gpsimd.dma_start(dbg["dk"][:], K[:])
            nc.gpsimd.dma_start(dbg["dvt"][:], VTO[:])
            nc.sync.dma_start(dbg["dy0"][:], y_all[0][:])
            nc.sync.dma_start(dbg["dy1"][:], y_all[1][:])
            nc.sync.dma_start(dbg["dot0"][:], ot[0][:].bitcast(F32))
            nc.sync.dma_start(dbg["dot1"][:], ot[1][:].bitcast(F32))
            nc.sync.dma_start(dbg["dvp0"][:], vp[0][:].bitcast(F32))
            nc.sync.dma_start(dbg["dvp1"][:], vp[1][:].bitcast(F32))


def make_in_maps(x_full, consts):
    maps = []
    for b in range(8):
        m = dict(consts)
        m["x"] = np.ascontiguousarray(x_full[b].reshape(256, HW), dtype=np.float32)
        maps.append(m)
    return maps

_CACHED = {}


def _get_nc():
    if 'nc' not in _CACHED:
        _CACHED['nc'] = build_nc(debug=False)
    return _CACHED['nc']


def kernel(**inputs):
    """Full (unsharded) inputs -> full output (8, 256, 48, 48) float32."""
    from concourse.bass_utils import run_bass_kernel_spmd

    x = np.asarray(inputs['x'], dtype=np.float32)
    consts = build_consts(**{k: np.asarray(v) for k, v in inputs.items()
                             if k != 'x'})
    in_maps = make_in_maps(x, consts)
    nc = _get_nc()
    try:
        res = run_bass_kernel_spmd(nc, in_maps, list(range(8)))
    except Exception:
        # first execution after a fresh compile occasionally hits a
        # transient device error; one retry clears it
        res = run_bass_kernel_spmd(nc, in_maps, list(range(8)))
    out = np.stack([res.results[b]['out'].reshape(256, 48, 48)
                    for b in range(8)])
    return out.astype(np.float32)
